# revision 4
# baseline (speedup 1.0000x reference)
"""Trainium2 Bass kernel for the MTGNN top-k adjacency masking problem.

Strategy (row-sharded across 8 NeuronCores):
  host:   n1/n2 node factors (tiny), t = fl(0.01 * noise)
  device: a = [n1|n2] @ [n2|-n1]^T (row block), adj = tanh(3a) (ScalarE),
          score = adj + t (bit-exact fp32 add folded into the noise DMA via
          SWDGE accum), per-row segment-max -> 32nd-largest segment max M32
          (provable lower bound on the 32nd-largest element), candidate map
          out = max(score, M32 - margin) - (M32 - margin)  (bf16, >0 mask)
  host:   exact top-32 per row among the ~34 device candidates, re-scored in
          CPU fp32 so selection/tie-break (value desc, index asc) matches the
          reference top_k bit-for-bit; values = relu(tanh(3a)).
"""
import os
import sys

import numpy as np

for _p in ("/opt/trn_rl_repo", os.path.expanduser("~/.axon_site/_ro/trn_rl_repo")):
    if os.path.isdir(_p) and _p not in sys.path:
        sys.path.insert(0, _p)

from concourse import bacc, mybir, tile  # noqa: E402
from concourse.bass_utils import run_bass_kernel_spmd  # noqa: E402

N = 8192
DIM = 64
K = 32
ALPHA = np.float32(3.0)
M = 8                    # cores
ROWS = N // M            # rows per core (1024)
P = 128                  # partitions
NRT = ROWS // P          # row tiles per core (8)
SEG = 32                 # segment width for the seg-max prune
NSEG = N // SEG          # 256 segments per row
MMW = 512                # matmul moving free dim (one PSUM bank fp32)
PSW = 2048               # psum group width (4 banks)
CTW = P * NRT            # 1024 columns of lhsT block
MARGIN = np.float32(3e-5)  # threshold slack, >> 4-ulp device tanh drift

f32 = mybir.dt.float32
bf16 = mybir.dt.bfloat16

_BUILT = None


def _build():
    nc = bacc.Bacc(None, target_bir_lowering=False, debug=False)
    # single DRAM input for both matmul operands: [128, CTW | N]
    cd_in = nc.declare_dram_parameter("cd", [P, CTW + N], f32, isOutput=False)
    t_in = nc.declare_dram_parameter("t", [ROWS, N], f32, isOutput=False)
    out_d = nc.declare_dram_parameter("out", [ROWS, N], bf16, isOutput=True)
    m32_d = nc.declare_dram_parameter("m32", [P, NRT], f32, isOutput=True)

    with tile.TileContext(nc) as tc:
        with (
            tc.tile_pool(name="const", bufs=1) as cpool,
            tc.tile_pool(name="score", bufs=2) as spool,
            tc.tile_pool(name="outp", bufs=2) as opool,
            tc.tile_pool(name="psum", bufs=2, space="PSUM") as ppool,
            tc.tile_pool(name="aux", bufs=2) as apool,
        ):
            cd = cpool.tile([P, CTW + N], f32)
            m32t = cpool.tile([P, NRT], f32)
            nc.sync.dma_start(out=cd[:], in_=cd_in[:])

            for rt in range(NRT):
                s = spool.tile([P, N], f32, tag="score")
                for cc in range(N // PSW):
                    ps = ppool.tile([P, PSW], f32, tag="ps")
                    for q in range(PSW // MMW):
                        j0 = PSW * cc + MMW * q
                        nc.tensor.matmul(
                            ps[:, MMW * q : MMW * (q + 1)],
                            lhsT=cd[:, P * rt : P * (rt + 1)],
                            rhs=cd[:, CTW + j0 : CTW + j0 + MMW],
                            start=True, stop=True,
                        )
                    nc.scalar.activation(
                        s[:, PSW * cc : PSW * (cc + 1)], ps[:],
                        mybir.ActivationFunctionType.Tanh, scale=float(ALPHA),
                    )
                # score = adj + t, bit-exact fp32 add during the noise load
                # (CCE accum descriptors are limited to 2048 elems/partition)
                for ac in range(0, N, PSW):
                    nc.gpsimd.dma_start(
                        out=s[:, ac : ac + PSW],
                        in_=t_in[P * rt : P * (rt + 1), ac : ac + PSW],
                        accum_op=mybir.AluOpType.add,
                    )
                # 32nd largest segment max
                sm = apool.tile([P, NSEG], f32, tag="sm")
                nc.vector.tensor_reduce(
                    sm[:], s[:].rearrange("p (g w) -> p g w", w=SEG),
                    mybir.AxisListType.X, mybir.AluOpType.max,
                )
                m8 = apool.tile([P, 8], f32, tag="m8")
                for r in range(4):
                    nc.vector.max(m8[:], sm[:])
                    if r < 3:
                        nc.vector.match_replace(sm[:], m8[:], sm[:], -1e30)
                nc.vector.tensor_scalar(
                    m32t[:, rt : rt + 1], m8[:, 7:8], float(MARGIN), None,
                    mybir.AluOpType.subtract,
                )
                # candidate map (bf16): positive iff score >= M32m
                o = opool.tile([P, N], bf16, tag="outp")
                nc.vector.tensor_scalar(
                    o[:], s[:], m32t[:, rt : rt + 1], m32t[:, rt : rt + 1],
                    mybir.AluOpType.max, mybir.AluOpType.subtract,
                )
                nc.sync.dma_start(out=out_d[P * rt : P * (rt + 1), :], in_=o[:])
            nc.sync.dma_start(out=m32_d[:], in_=m32t[:])
    nc.compile()
    return nc


def _host_prep(idx, emb1_w, emb2_w, w1, b1, w2, b2, noise):
    idx = np.asarray(idx)
    e1 = np.asarray(emb1_w, dtype=np.float32)[idx]
    e2 = np.asarray(emb2_w, dtype=np.float32)[idx]
    w1 = np.asarray(w1, dtype=np.float32)
    b1 = np.asarray(b1, dtype=np.float32)
    w2 = np.asarray(w2, dtype=np.float32)
    b2 = np.asarray(b2, dtype=np.float32)
    n1 = np.tanh(ALPHA * (e1 @ w1.T + b1)).astype(np.float32)
    n2 = np.tanh(ALPHA * (e2 @ w2.T + b2)).astype(np.float32)
    C = np.concatenate([n1, n2], axis=1).astype(np.float32)        # [N, 128]
    D = np.concatenate([n2, -n1], axis=1).astype(np.float32).T     # [128, N]
    D = np.ascontiguousarray(D)
    t = (np.asarray(noise, dtype=np.float32) * np.float32(0.01)).astype(np.float32)
    return C, D, t


def _run_device(C, D, t, trace=False):
    global _BUILT
    if _BUILT is None:
        _BUILT = _build()
    nc = _BUILT
    in_maps = []
    for c in range(M):
        r0 = c * ROWS
        cd = np.concatenate(
            [np.ascontiguousarray(C[r0 : r0 + ROWS].T), D], axis=1
        ).astype(np.float32)
        in_maps.append({"cd": cd, "t": np.ascontiguousarray(t[r0 : r0 + ROWS])})
    res = run_bass_kernel_spmd(nc, in_maps, list(range(M)), trace=trace)
    cand = np.empty((N, N), dtype=np.uint8)
    m32m = np.empty(N, dtype=np.float32)
    for c in range(M):
        r0 = c * ROWS
        ob = res.results[c]["out"]          # [ROWS, N] bf16
        cand[r0 : r0 + ROWS] = (ob.view(np.uint16) != 0).view(np.uint8)
        m32m[r0 : r0 + ROWS] = res.results[c]["m32"].T.reshape(-1)
    return cand, m32m, res


def _host_trim(C, D, t, cand, m32m):
    """Exact per-row top-32 among device candidates, CPU-scored."""
    out = np.zeros((N, N), dtype=np.float32)
    rows, cols = np.nonzero(cand)
    # CPU re-score of candidates
    a = np.einsum("nk,kn->n", C[rows], D[:, cols], dtype=np.float32).astype(np.float32)
    adj = np.maximum(np.tanh(ALPHA * a).astype(np.float32), np.float32(0.0))
    sc = (adj + t[rows, cols]).astype(np.float32)

    counts = np.bincount(rows, minlength=N)
    bad_rows = set(np.nonzero((counts < K) | (m32m < np.float32(0.011)))[0].tolist())

    order = np.lexsort((cols, -sc.astype(np.float64), rows))
    rows_s, cols_s, adj_s = rows[order], cols[order], adj[order]
    starts = np.zeros(N + 1, dtype=np.int64)
    np.cumsum(counts, out=starts[1:])
    pos = np.arange(len(rows_s)) - starts[rows_s]
    sel = pos < K
    out[rows_s[sel], cols_s[sel]] = adj_s[sel]

    for r in bad_rows:  # exact fallback, exceedingly rare (verified empty)
        a_r = (C[r : r + 1] @ D).astype(np.float32).reshape(-1)
        adj_r = np.maximum(np.tanh(ALPHA * a_r).astype(np.float32), np.float32(0.0))
        sc_r = (adj_r + t[r]).astype(np.float32)
        o = np.lexsort((np.arange(N), -sc_r.astype(np.float64)))[:K]
        out[r] = 0.0
        out[r, o] = adj_r[o]
    return out


def kernel(idx, emb1_w, emb2_w, w1, b1, w2, b2, noise):
    C, D, t = _host_prep(idx, emb1_w, emb2_w, w1, b1, w2, b2, noise)
    cand, m32m, _ = _run_device(C, D, t, trace=False)
    return _host_trim(C, D, t, cand, m32m)


def kernel_profiled(idx, emb1_w, emb2_w, w1, b1, w2, b2, noise):
    """Same as kernel() but returns (out, BassKernelResults-with-profile)."""
    C, D, t = _host_prep(idx, emb1_w, emb2_w, w1, b1, w2, b2, noise)
    cand, m32m, res = _run_device(C, D, t, trace=True)
    return _host_trim(C, D, t, cand, m32m), res
